# revision 1
# baseline (speedup 1.0000x reference)
"""Trainium2 Bass kernel for nn_ConcaveNN (UMNN-style nested double quadrature).

Math restructure — Fubini order swap (validated vs the jax reference to
~5e-5 rel err at n=26 on the actual seed-0 inputs; HW f32r/bf16 noise
brings the end-to-end error to ~1e-3 vs the 2e-2 gate):

  The reference nests CC quadrature: pos = Q_t[ Q_u over [t,T] g_p ],
  neg = Q_t[ -Q_u over [0,t] g_n ] — 2*51*51 MLP evals per sample.
  Swapping the order of integration analytically:

    pos = I u g_p(u) du over [0,x]  +  x * I g_p(u) du over [x,T]
    neg = -I (x-u) g_n(u) du over [0,x]

  so one n=26 CC rule per single integral needs only 3*27 = 81 MLP
  evals per sample (vs 5202).

Per-core layout (16 samples, pure data parallel across 8 cores):
  point stream [1296] = pos(864: 16 samples x 54 (27 A + 27 B pts))
                      | neg(432: 16 x 27); 3 matmul tiles of 432.

  L1: one K=17 f32r matmul per tile, lhsT17 = [w0row0; c_0..c_15] with
  c_i = b0 + h_i @ W0[1:] host-precomputed; rhs17 row 0 = u (tiny DMA),
  rows 1..16 = sample one-hot built ON DEVICE (iota + is_equal on
  GpSimd) so no 80KB one-hot DMA. L2 bf16. L3 = w2 tiled 32-wide bf16,
  tile_position-packed at partition offsets 0/32 (pos) / 64 (neg) into
  ONE PSUM bank; all 32 partitions of a group hold identical y rows.
  A psum->sbuf copy adds b2 (per-partition bias), then two DMAs fold
  rows {0,32,64} into a dense per-sample [16, 81] tile (sample i row =
  [posA|posB|negA] rule values), so the elu tail + fused quadrature
  dot (V precomputed on host) directly yields per-sample sums — no
  partition->sample matmul needed.

  elu(z)+1 = max(z,0) + min(exp(z),1) (z = y + b2).
"""
import sys

import ml_dtypes
import numpy as np

sys.path.insert(0, "/opt/trn_rl_repo")

import concourse.bass as bass  # noqa: E402
import concourse.mybir as mybir  # noqa: E402
import concourse.tile as tile  # noqa: E402
from contextlib import ExitStack  # noqa: E402
from concourse import bacc  # noqa: E402
from concourse.bass_utils import run_bass_kernel_spmd  # noqa: E402
from concourse.tile import add_dep_helper  # noqa: E402

F32 = mybir.dt.float32
F32R = mybir.dt.float32r
BF16 = mybir.dt.bfloat16
I32 = mybir.dt.int32

B, DH, HID = 128, 32, 128
NCORES = 8
SPC = B // NCORES                # 16 samples per core
NQ = 26                          # CC order for the swapped single integrals
N1 = NQ + 1                      # 27 points per rule
PPS = 2 * N1                     # 54 pos points per sample (A + B)
TILE = 8 * PPS                   # 432 = one matmul tile (8 pos samples)
NPOS = SPC * PPS                 # 864
NNEG = SPC * N1                  # 432
NTOT = NPOS + NNEG               # 1296
NR = 3 * N1                      # 81 = per-sample fold row

_CACHE = {}


def _cc_consts(n):
    lam = np.arange(n + 1, dtype=np.float64).reshape(-1, 1)
    lam = np.cos(lam @ lam.T * np.pi / n)
    lam[:, 0] = 0.5
    lam[:, -1] = 0.5 * lam[:, -1]
    lam = lam * 2.0 / n
    W = np.arange(n + 1, dtype=np.float64).reshape(-1, 1)
    W[1::2] = 0.0
    W = 2.0 / (1.0 - W**2)
    W[0] = 1.0
    W[1::2] = 0.0
    ccw = (lam.T @ W)[:, 0]
    a = (np.cos(np.arange(n + 1, dtype=np.float64) * np.pi / n) + 1.0) * 0.5
    return ccw, a


def _build_module():
    nc = bacc.Bacc(
        "TRN2", target_bir_lowering=False, debug=False, num_devices=NCORES
    )

    def din(name, shape, dtype=F32):
        return nc.dram_tensor(name, shape, dtype, kind="ExternalInput").ap()

    ul_ap = din("ul", [1, NTOT + 256], F32R)      # u points | w0row0 pair
    oc_ap = din("oc", [16, NTOT + 256], BF16)     # sample one-hot | C pair
    wb_ap = din("wb", [128, 448], BF16)           # w1p | w1n | cw1 | w2 x32
    wf_ap = din("wf", [128, 234], F32)            # biases, V, cb2, cw2, head
    out_ap = nc.dram_tensor("out", [SPC, 1], F32, kind="ExternalOutput").ap()

    AF = mybir.ActivationFunctionType
    OP = mybir.AluOpType
    GP = mybir.EngineType.Pool

    with tile.TileContext(nc) as tc, ExitStack() as ctx:
        const = ctx.enter_context(tc.tile_pool(name="const", bufs=1))
        z1p = ctx.enter_context(tc.tile_pool(name="z1p", bufs=3))
        z2p = ctx.enter_context(tc.tile_pool(name="z2p", bufs=3))
        tp = ctx.enter_context(tc.tile_pool(name="tp", bufs=1))
        p1 = ctx.enter_context(tc.tile_pool(name="p1", bufs=3, space="PSUM"))
        p2 = ctx.enter_context(tc.tile_pool(name="p2", bufs=2, space="PSUM"))
        p3 = ctx.enter_context(tc.tile_pool(name="p3", bufs=1, space="PSUM"))
        pm = ctx.enter_context(tc.tile_pool(name="pm", bufs=1, space="PSUM"))

        # ---- input DMAs spread across the three DGE-capable engines
        # (SP, ACT, Pool) so descriptor-gen overlaps; ACT then preloads
        # the activation-function table (1.3us) under the transfers ----
        ul = const.tile_from(ul_ap, name="ul")
        oc = const.tile_from(oc_ap, name="oc",
                             forced_dma_engine=mybir.EngineType.Activation)
        wb = const.tile_from(wb_ap, name="wb", forced_dma_engine=GP)
        wf = const.tile_from(wf_ap, name="wf")

        dum = tp.tile([1, 1], F32, tag="dum")
        nc.vector.memset(dum[:], 0.0)
        dum2 = tp.tile([1, 1], F32, tag="dum2")
        nc.scalar.activation(dum2[:], dum[:], AF.Exp)
        yf = tp.tile([SPC, NR], F32, tag="yf")
        nc.vector.memset(yf[:], 0.0)

        u_sb = ul[:, 0:NTOT]
        lhsTa = [ul[:, NTOT:NTOT + 128], ul[:, NTOT + 128:NTOT + 256]]
        oh_sb = oc[:, 0:NTOT]
        lhsTc = [oc[:, NTOT:NTOT + 128], oc[:, NTOT + 128:NTOT + 256]]
        w1 = [wb[:, 0:128], wb[:, 128:256]]
        cw1 = wb[:, 256:384]
        w2 = [wb[:, 384:416], wb[:, 416:448]]
        b1 = [wf[:, 0:1], wf[:, 1:2]]
        b2c = wf[:, 2:3]
        cb1 = wf[:, 3:4]
        v_fold = wf[0:SPC, 4:4 + NR]
        cb2 = wf[0:SPC, 85:87]
        cw2 = wf[:, 87:89]
        haug, cw0m = wf[0:33, 90:106], wf[0:33, 106:234]

        NETOF = (0, 0, 1)  # net per tile

        # ---- L1 per tile: rank-1 f32r matmul (a*u) + K=16 bf16 matmul
        # (one-hot picks the per-sample bias row c_i), accumulated;
        # grouped by stationary operand to minimize weight reloads ----
        z1 = []
        pts = [p1.tile([128, TILE], F32, tag="p1", name=f"p1_{t}")
               for t in range(3)]
        zts = [z1p.tile([128, TILE], BF16, tag="z1", name=f"z1_{t}")
               for t in range(3)]

        def sl(t):
            return slice(TILE * t, TILE * (t + 1))

        for t in (0, 1):
            nc.tensor.matmul(pts[t][:], lhsT=lhsTa[0], rhs=u_sb[:, sl(t)],
                             start=True, stop=False)
        mmb0 = None
        for t in (0, 1):
            mi = nc.tensor.matmul(pts[t][:], lhsT=lhsTc[0],
                                  rhs=oh_sb[:, sl(t)],
                                  start=False, stop=True)
            if t == 0:
                mmb0 = mi
            if t == 1:
                nc.vector.tensor_scalar_max(zts[t][:], pts[t][:], 0.0)
            else:
                nc.scalar.activation(zts[t][:], pts[t][:], AF.Relu)
        nc.tensor.matmul(pts[2][:], lhsT=lhsTa[1], rhs=u_sb[:, sl(2)],
                         start=True, stop=False)
        nc.tensor.matmul(pts[2][:], lhsT=lhsTc[1], rhs=oh_sb[:, sl(2)],
                         start=False, stop=True)
        nc.scalar.activation(zts[2][:], pts[2][:], AF.Relu)
        z1 = zts

        # ---- L2 + relu(+b1); head MLP matmuls interleaved to fill PE ----
        z2 = []
        for t in range(3):
            pt = p2.tile([128, TILE], F32, tag="p2", name=f"p2_{t}")
            nc.tensor.matmul(pt[:], lhsT=w1[NETOF[t]],
                             rhs=z1[t][:], start=True, stop=True)
            zt = z2p.tile([128, TILE], BF16, tag="z2", name=f"z2_{t}")
            bias = b1[NETOF[t]]
            if t == 1:
                nc.scalar.activation(zt[:], pt[:], AF.Relu, bias=bias)
            else:
                nc.vector.tensor_scalar(zt[:], pt[:], bias, 0.0, OP.add, OP.max)
            z2.append(zt)
            if t == 0:
                ph1 = pm.tile([128, SPC], F32, tag="pm", name="ph1")
                mi = nc.tensor.matmul(ph1[:], lhsT=cw0m, rhs=haug,
                                      start=True, stop=True)
                # keep the head matmul behind L1 in PE's stream so a
                # late wf DMA cannot stall the main pipeline
                add_dep_helper(mi.ins, mmb0.ins, sync=False,
                               reason="head after L1")
                z1h = tp.tile([128, SPC], BF16, tag="z1h")
                nc.scalar.activation(z1h[:], ph1[:], AF.Relu)
            elif t == 1:
                ph2 = pm.tile([128, SPC], F32, tag="pm", name="ph2")
                nc.tensor.matmul(ph2[:], lhsT=cw1, rhs=z1h[:],
                                 start=True, stop=True)
                z2h = tp.tile([128, SPC], F32, tag="z2h")
                nc.scalar.activation(z2h[:], ph2[:], AF.Relu, bias=cb1)

        # ---- L3: w2 (x32) bf16, tile_position-packed into ONE bank:
        # pos tiles at partition offsets 0/32, neg at 64 ----
        bank = p3.tile([96, TILE], F32, tag="p3", name="bank")
        for t in range(2):
            nc.tensor.matmul(bank[32 * t: 32 * t + 32, :], lhsT=w2[0],
                             rhs=z2[t][:], start=True, stop=True,
                             tile_position=(0, 32 * t))
        ph3 = pm.tile([SPC, 2], F32, tag="pm", name="ph3")
        nc.tensor.matmul(ph3[:], lhsT=z2h[:], rhs=cw2, start=True, stop=True)
        oh = tp.tile([SPC, 2], F32, tag="oh")
        nc.vector.tensor_add(oh[:], ph3[:], cb2)
        sc = tp.tile([SPC, 1], F32, tag="sc")
        nc.scalar.activation(sc[:], oh[:, 1:2], AF.Exp)
        nc.tensor.matmul(bank[64:96, :], lhsT=w2[1], rhs=z2[2][:],
                         start=True, stop=True, tile_position=(0, 64))

        # ---- psum->sbuf with per-partition b2 added (pos half first so
        # its fold DMA overlaps the neg L3 matmul), then fold rows
        # {0,32,64} into per-sample [16, 81] = [posA|posB|negA] ----
        yc = tp.tile([96, TILE], F32, tag="yc")
        nc.scalar.activation(yc[0:64, :], bank[0:64, :], AF.Identity,
                             bias=b2c[0:64, 0:1])
        nc.sync.dma_start(out=yf[:, 0:PPS], in_=yc[0:33:32, :])
        nc.vector.tensor_scalar_add(yc[64:96, :], bank[64:96, :],
                                    b2c[64:96, 0:1])
        nc.scalar.dma_start(out=yf[:, PPS:NR], in_=yc[64:65, :])

        # ---- elu tail: elu(z)+1 = max(z,0) + min(exp(z),1); dot V ----
        e = tp.tile([SPC, NR], F32, tag="e")
        nc.scalar.activation(e[:], yf[:], AF.Exp)
        r = tp.tile([SPC, NR], F32, tag="r")
        nc.vector.tensor_scalar_max(r[:], yf[:], 0.0)
        s = tp.tile([SPC, NR], F32, tag="s")
        nc.vector.scalar_tensor_tensor(s[:], e[:], 1.0, r[:], OP.min, OP.add)
        rv = tp.tile([SPC, NR], F32, tag="rv")
        nc.vector.tensor_mul(rv[:], s[:], v_fold)
        rs = tp.tile([SPC, 1], F32, tag="rs")
        nc.vector.tensor_reduce(rs[:], rv[:], mybir.AxisListType.X, OP.add)

        # ---- final combine: out = rs * scaling + offset ----
        out_sb = tp.tile([SPC, 1], F32, tag="outsb")
        nc.vector.tensor_scalar(out_sb[:], rs[:], sc[:, 0:1], oh[:, 0:1],
                                OP.mult, OP.add)
        nc.sync.dma_start(out=out_ap[:], in_=out_sb[:])

    nc.compile()
    return nc


def _get_module():
    if "nc" not in _CACHE:
        _CACHE["nc"] = _build_module()
    return _CACHE["nc"]


def make_in_maps(**inputs):
    """Host-side prep: quadrature points/weights + packed param tensors."""
    f = lambda k: np.asarray(inputs[k], np.float64)
    f32 = lambda k: np.asarray(inputs[k], np.float32)
    bf16 = ml_dtypes.bfloat16
    x_full = f("x")                                      # [B,1]
    h_full = f("h")
    ccw, a = _cc_consts(NQ)                              # f64 [27]
    T = np.float64(np.float32(x_full.max()) + np.float32(10.0))

    wb = np.zeros((128, 448), bf16)
    wb[:, 0:128] = f32("pw1").astype(bf16)
    wb[:, 128:256] = f32("nw1").astype(bf16)
    wb[:, 256:384] = f32("cw1").astype(bf16)
    wb[:, 384:416] = np.tile(f32("pw2"), (1, 32)).astype(bf16)
    wb[:, 416:448] = np.tile(f32("nw2"), (1, 32)).astype(bf16)

    # one-hot sample-selector rows (constant pattern)
    oh = np.zeros((16, NTOT), np.float32)
    for i in range(SPC):
        oh[i, PPS * i: PPS * (i + 1)] = 1.0
        oh[i, NPOS + N1 * i: NPOS + N1 * (i + 1)] = 1.0

    in_maps = []
    for c in range(NCORES):
        sl = slice(SPC * c, SPC * (c + 1))
        x = x_full[sl, 0]                                # [16]
        h = h_full[sl]                                   # [16,32]

        uA = x[:, None] * a[None, :]                     # [16,27]
        uB = x[:, None] + (T - x[:, None]) * a[None, :]
        vA = ccw[None, :] * uA * (x[:, None] / 2.0)      # pos, du part
        vBw = ccw[None, :] * (x[:, None] * (T - x[:, None]) / 2.0)
        vN = -ccw[None, :] * (1.0 - a[None, :]) * (x[:, None] ** 2 / 2.0)

        ul = np.zeros((1, NTOT + 256), np.float32)
        ul[0, 0:NPOS] = np.concatenate(
            [uA, uB], 1).reshape(-1).astype(np.float32)
        ul[0, NPOS:NTOT] = uA.reshape(-1).astype(np.float32)
        oc = np.zeros((16, NTOT + 256), bf16)
        oc[:, 0:NTOT] = oh.astype(bf16)
        for k, p in enumerate("pn"):
            w0, b0 = f32(p + "w0"), f32(p + "b0")
            ul[0, NTOT + 128 * k: NTOT + 128 * k + 128] = w0[0]
            oc[:, NTOT + 128 * k: NTOT + 128 * k + 128] = (
                b0[None, :] + h.astype(np.float32) @ w0[1:, :]).astype(bf16)

        wf = np.zeros((128, 234), np.float32)
        wf[:, 0] = f32("pb1")
        wf[:, 1] = f32("nb1")
        wf[0:64, 2] = f32("pb2")[0]
        wf[64:96, 2] = f32("nb2")[0]
        wf[:, 3] = f32("cb1")
        vpos = np.concatenate([vA, vBw], 1)              # [16, 54]
        wf[0:SPC, 4:4 + PPS] = vpos.astype(np.float32)
        wf[0:SPC, 4 + PPS:4 + NR] = vN.astype(np.float32)
        wf[0:SPC, 85:87] = np.tile(f32("cb2")[None, :], (SPC, 1))
        wf[:, 87:89] = f32("cw2")
        wf[0, 90:106] = 1.0
        wf[1:33, 90:106] = h.T.astype(np.float32)
        wf[0, 106:234] = f32("cb0")
        wf[1:33, 106:234] = f32("cw0")

        in_maps.append(dict(ul=ul, oc=oc, wb=wb, wf=wf))
    return in_maps


def kernel(**inputs):
    nc = _get_module()
    in_maps = make_in_maps(**inputs)
    res = run_bass_kernel_spmd(nc, in_maps, list(range(NCORES)))
    out = np.concatenate([res.results[c]["out"] for c in range(NCORES)], 0)
    return out.astype(np.float32)


if __name__ == "__main__":
    rng = np.random.default_rng(0)
    ins = dict(
        x=rng.random((B, 1), np.float32) * 2.0,
        h=rng.standard_normal((B, DH)).astype(np.float32),
    )
    for p in "pn":
        ins[p + "w0"] = rng.standard_normal((DH + 1, HID)).astype(np.float32) * 0.1
        ins[p + "b0"] = rng.standard_normal((HID,)).astype(np.float32) * 0.1
        ins[p + "w1"] = rng.standard_normal((HID, HID)).astype(np.float32) * 0.1
        ins[p + "b1"] = rng.standard_normal((HID,)).astype(np.float32) * 0.1
        ins[p + "w2"] = rng.standard_normal((HID, 1)).astype(np.float32) * 0.1
        ins[p + "b2"] = rng.standard_normal((1,)).astype(np.float32) * 0.1
    ins["cw0"] = rng.standard_normal((DH, HID)).astype(np.float32) * 0.1
    ins["cb0"] = rng.standard_normal((HID,)).astype(np.float32) * 0.1
    ins["cw1"] = rng.standard_normal((HID, HID)).astype(np.float32) * 0.1
    ins["cb1"] = rng.standard_normal((HID,)).astype(np.float32) * 0.1
    ins["cw2"] = rng.standard_normal((HID, 2)).astype(np.float32) * 0.1
    ins["cb2"] = rng.standard_normal((2,)).astype(np.float32) * 0.1
    print(kernel(**ins)[:4, 0])



# revision 2
# speedup vs baseline: 1.1185x; 1.1185x over previous
"""Trainium2 Bass kernel for nn_ConcaveNN (UMNN-style nested double quadrature).

Math restructure — Fubini order swap (validated vs the jax reference on the
actual seed-0 inputs):

  pos = I u g_p(u) du over [0,x]  +  x * I g_p(u) du over [x,T]
  neg = -I (x-u) g_n(u) du over [0,x]

Quadrature: Gauss-Legendre, orders (A=4, B=8, N=4) per sample -> 16 MLP
points per sample (vs reference's 5202).  Scheme error on seed-0 inputs:
5.2e-3 abs vs a 0.449 abs budget (rel gate 2e-2 * max|out| 22.4).

Per-core layout (16 samples, pure data parallel across 8 cores):
  ONE 256-column point stream: cols 0:192 = pos points sample-major
  (12/sample = A4|B8), cols 192:256 = neg (4/sample).  All three MLP
  layers run on this single tile:

  L1: ONE K=34 f32r matmul. rhs34 = [u*maskpos; onehot_pos; u*maskneg;
  onehot_neg], lhsT34 = [pw0row0; Cp; nw0row0; Cn] with C = b0 + h@W0[1:]
  host-precomputed (f32 now, not bf16).  The masks zero cross-terms, so
  pos columns get net-p and neg columns net-n in one pass.
  L2/L3: per-net column-range matmuls (bf16).  L3 uses M=1 lhsT (w2), so
  the y-stream lands on PSUM partition 0 directly — no 32x replication,
  no partition-fold DMAs (the old version burned ~2us of DMA latency
  re-laying out [96,432] -> [16,81]).

  Tail (all partition 0, no cross-partition moves): elu(z)+1 =
  max(z+b2,0) + min(exp(z+b2),1) via ACT exp + DVE max + one
  scalar_tensor_tensor; multiply by host-precomputed fused quadrature
  weights V while scattering to sample-major; ONE windowed tensor_reduce
  [1,(16,16)] -> [1,16] gives per-sample integrals.  Head runs
  transposed (two M=1 matmuls -> [1,32] PSUM row), so scaling/offset
  combine happens on partition 0 as well; output is a single 64B DMA.

  DMA plan: critical L1 operands on Pool (shortest DGE chain), L2/L3
  weights on ACT (desc-gen first, then the exp-table preload), the rest
  on SP.  Output DMA on Pool.
"""
import sys

import ml_dtypes
import numpy as np

sys.path.insert(0, "/opt/trn_rl_repo")

import concourse.bass as bass  # noqa: E402
import concourse.mybir as mybir  # noqa: E402
import concourse.tile as tile  # noqa: E402
from contextlib import ExitStack  # noqa: E402
from concourse import bacc  # noqa: E402
from concourse.bass_utils import run_bass_kernel_spmd  # noqa: E402

F32 = mybir.dt.float32
F32R = mybir.dt.float32r
BF16 = mybir.dt.bfloat16

B, DH, HID = 128, 32, 128
NCORES = 8
SPC = B // NCORES                # 16 samples per core
NA, NB, NN = 4, 8, 4            # GL orders: A/[0,x], B/[x,T], N/[0,x]
PP = NA + NB                    # 12 pos points per sample
PW = PP + NN                    # 16 points per sample
POSW = SPC * PP                 # 192 pos columns
NEGW = SPC * NN                 # 64 neg columns
NCOL = POSW + NEGW              # 256 total columns

_CACHE = {}


def _gl(n):
    xn, wn = np.polynomial.legendre.leggauss(n)
    return wn / 2.0, (xn + 1.0) / 2.0  # weights/nodes on [0,1]


def _build_module():
    nc = bacc.Bacc(
        "TRN2", target_bir_lowering=False, debug=False, num_devices=NCORES
    )

    def din(name, shape, dtype=F32):
        return nc.dram_tensor(name, shape, dtype, kind="ExternalInput").ap()

    cr_ap = din("cr", [34, 384], F32R)      # rhs34 (256) | lhsT34 (128)
    wbf_ap = din("wbf", [128, 388], BF16)   # pw1|nw1|cw1|pw2|nw2|cw2
    wsm_ap = din("wsm", [128, 408], F32)    # biases, head, V row
    out_ap = nc.dram_tensor("out", [SPC, 1], F32, kind="ExternalOutput").ap()

    AF = mybir.ActivationFunctionType
    OP = mybir.AluOpType
    AX = mybir.AxisListType

    with tile.TileContext(nc) as tc, ExitStack() as ctx:
        const = ctx.enter_context(tc.tile_pool(name="const", bufs=1))
        tp = ctx.enter_context(tc.tile_pool(name="tp", bufs=1))
        pA = ctx.enter_context(tc.tile_pool(name="pA", bufs=1, space="PSUM"))
        pB = ctx.enter_context(tc.tile_pool(name="pB", bufs=1, space="PSUM"))
        pC = ctx.enter_context(tc.tile_pool(name="pC", bufs=1, space="PSUM"))
        pH1 = ctx.enter_context(tc.tile_pool(name="pH1", bufs=1, space="PSUM"))
        pH2 = ctx.enter_context(tc.tile_pool(name="pH2", bufs=1, space="PSUM"))
        pT = ctx.enter_context(tc.tile_pool(name="pT", bufs=1, space="PSUM"))

        # ---- input DMAs: critical L1 tile on Pool (shortest DGE chain),
        # L2/L3 weights on ACT before its table preload, rest on SP ----
        cr = const.tile_from(cr_ap, name="cr",
                             forced_dma_engine=mybir.EngineType.Pool)
        wbf = const.tile_from(wbf_ap, name="wbf",
                              forced_dma_engine=mybir.EngineType.Activation)
        wsm = const.tile_from(wsm_ap, name="wsm")

        # ACT exp-table preload off the framework const-zero AP (no deps)
        dum = tp.tile([1, 1], F32, tag="dum")
        zap = nc.const_aps.aps[(mybir.dt.float32, 0.0)]
        nc.scalar.activation(dum[:], zap[0:1, 0:1], AF.Exp)

        rhs34 = cr[:, 0:NCOL]
        lhsT34 = cr[:, NCOL:NCOL + 128]
        w1p, w1n, cw1 = wbf[:, 0:128], wbf[:, 128:256], wbf[:, 256:384]
        w2p, w2n = wbf[:, 384:385], wbf[:, 385:386]
        cw2o, cw2s = wbf[:, 386:387], wbf[:, 387:388]
        pb1, nb1, cb1 = wsm[:, 0:1], wsm[:, 1:2], wsm[:, 2:3]
        pb2, nb2 = wsm[0:1, 3:4], wsm[0:1, 4:5]
        cb2o, cb2s = wsm[0:1, 5:6], wsm[0:1, 6:7]
        haug = wsm[0:33, 8:24]
        cw0m = wsm[0:33, 24:152]
        vrow = wsm[0:1, 152:408]            # segregated: vpos | vneg

        # ---- L1: one K=34 f32r matmul (masked nets share the pass) ----
        pL1 = pA.tile([128, NCOL], F32, tag="pL1")
        nc.tensor.matmul(pL1[:], lhsT=lhsT34, rhs=rhs34,
                         start=True, stop=True)
        # head L1 fills PE while relu1 runs
        ph1 = pH1.tile([128, SPC], F32, tag="ph1")
        nc.tensor.matmul(ph1[:], lhsT=cw0m, rhs=haug, start=True, stop=True)

        z1 = tp.tile([128, NCOL], BF16, tag="z1")
        nc.scalar.activation(z1[:, 0:POSW], pL1[:, 0:POSW], AF.Relu)
        nc.vector.tensor_scalar_max(z1[:, POSW:NCOL], pL1[:, POSW:NCOL], 0.0)
        z1h = tp.tile([128, SPC], BF16, tag="z1h")
        nc.scalar.activation(z1h[:], ph1[:], AF.Relu)

        # ---- L2 (+b1 relu), per net ----
        pL2 = pB.tile([128, NCOL], F32, tag="pL2")
        nc.tensor.matmul(pL2[:, 0:POSW], lhsT=w1p, rhs=z1[:, 0:POSW],
                         start=True, stop=True)
        nc.tensor.matmul(pL2[:, POSW:NCOL], lhsT=w1n, rhs=z1[:, POSW:NCOL],
                         start=True, stop=True)
        ph2 = pH2.tile([128, SPC], F32, tag="ph2")
        nc.tensor.matmul(ph2[:], lhsT=cw1, rhs=z1h[:], start=True, stop=True)

        z2 = tp.tile([128, NCOL], BF16, tag="z2")
        nc.scalar.activation(z2[:, 0:POSW], pL2[:, 0:POSW], AF.Relu, bias=pb1)
        nc.vector.tensor_scalar(z2[:, POSW:NCOL], pL2[:, POSW:NCOL],
                                nb1, 0.0, OP.add, OP.max)
        z2h = tp.tile([128, SPC], BF16, tag="z2h")
        nc.scalar.activation(z2h[:], ph2[:], AF.Relu, bias=cb1)

        # ---- L3: M=1 -> y stream on PSUM partition 0 ----
        pL3 = pC.tile([1, NCOL], F32, tag="pL3")
        nc.tensor.matmul(pL3[0:1, 0:POSW], lhsT=w2p, rhs=z2[:, 0:POSW],
                         start=True, stop=True)
        nc.tensor.matmul(pL3[0:1, POSW:NCOL], lhsT=w2n, rhs=z2[:, POSW:NCOL],
                         start=True, stop=True)
        # head L3, transposed: offset|presc as [1,32] on partition 0
        pHT = pT.tile([1, 2 * SPC], F32, tag="pHT")
        nc.tensor.matmul(pHT[0:1, 0:SPC], lhsT=cw2o, rhs=z2h[:],
                         start=True, stop=True)
        nc.tensor.matmul(pHT[0:1, SPC:2 * SPC], lhsT=cw2s, rhs=z2h[:],
                         start=True, stop=True)

        # ---- elu tail: s = max(z+b2,0) + min(exp(z+b2),1) ----
        e = tp.tile([1, NCOL], F32, tag="e")
        nc.scalar.activation(e[:, 0:POSW], pL3[0:1, 0:POSW], AF.Exp, bias=pb2)
        nc.scalar.activation(e[:, POSW:NCOL], pL3[0:1, POSW:NCOL], AF.Exp,
                             bias=nb2)
        r = tp.tile([1, NCOL], F32, tag="r")
        nc.vector.tensor_scalar(r[:, 0:POSW], pL3[0:1, 0:POSW],
                                pb2, 0.0, OP.add, OP.max)
        nc.vector.tensor_scalar(r[:, POSW:NCOL], pL3[0:1, POSW:NCOL],
                                nb2, 0.0, OP.add, OP.max)
        s = tp.tile([1, NCOL], F32, tag="s")
        nc.vector.scalar_tensor_tensor(s[:], e[:], 1.0, r[:], OP.min, OP.add)

        # ---- multiply by V, scattering to sample-major; windowed reduce ----
        sv = tp.tile([1, NCOL], F32, tag="sv")
        svp = sv[:].rearrange("p (s w) -> p s w", w=PW)[:, :, 0:PP]
        svn = sv[:].rearrange("p (s w) -> p s w", w=PW)[:, :, PP:PW]
        nc.vector.tensor_mul(svp, s[:, 0:POSW], vrow[:, 0:POSW])
        nc.vector.tensor_mul(svn, s[:, POSW:NCOL], vrow[:, POSW:NCOL])
        red = tp.tile([1, SPC], F32, tag="red")
        nc.vector.tensor_reduce(red[:], sv[:].rearrange("p (s w) -> p s w",
                                                        w=PW), AX.X, OP.add)

        # ---- combine: out = red * exp(presc+cb2s) + (offset+cb2o) ----
        sc = tp.tile([1, SPC], F32, tag="sc")
        nc.scalar.activation(sc[:], pHT[0:1, SPC:2 * SPC], AF.Exp, bias=cb2s)
        t1 = tp.tile([1, SPC], F32, tag="t1")
        nc.vector.tensor_mul(t1[:], red[:], sc[:])
        outsb = tp.tile([1, SPC], F32, tag="outsb")
        nc.vector.scalar_tensor_tensor(outsb[:], pHT[0:1, 0:SPC], cb2o,
                                       t1[:], OP.add, OP.add)
        nc.gpsimd.dma_start(out=out_ap[:], in_=outsb[:])

    nc.compile()
    return nc


def _get_module():
    if "nc" not in _CACHE:
        _CACHE["nc"] = _build_module()
    return _CACHE["nc"]


def make_in_maps(**inputs):
    """Host-side prep: quadrature points/weights + packed param tensors."""
    f = lambda k: np.asarray(inputs[k], np.float64)
    f32 = lambda k: np.asarray(inputs[k], np.float32)
    bf16 = ml_dtypes.bfloat16
    x_full = f("x")                                      # [B,1]
    h_full = f("h")
    wA, aA = _gl(NA)
    wB, aB = _gl(NB)
    wN, aN = _gl(NN)
    T = np.float64(np.float32(x_full.max()) + np.float32(10.0))

    wbf = np.zeros((128, 388), bf16)
    wbf[:, 0:128] = f32("pw1").astype(bf16)
    wbf[:, 128:256] = f32("nw1").astype(bf16)
    wbf[:, 256:384] = f32("cw1").astype(bf16)
    wbf[:, 384:385] = f32("pw2").astype(bf16)
    wbf[:, 385:386] = f32("nw2").astype(bf16)
    wbf[:, 386:388] = f32("cw2").astype(bf16)

    in_maps = []
    for c in range(NCORES):
        sl = slice(SPC * c, SPC * (c + 1))
        x = x_full[sl, 0]                                # [16]
        h = h_full[sl]                                   # [16,32]

        uA = x[:, None] * aA[None, :]                    # [16,4]
        uB = x[:, None] + (T - x[:, None]) * aB[None, :]  # [16,8]
        uN = x[:, None] * aN[None, :]                    # [16,4]
        vA = (x[:, None] * wA[None, :]) * uA             # weight u
        vB = ((T - x[:, None]) * wB[None, :]) * x[:, None]  # weight x
        vN = -(x[:, None] * wN[None, :]) * (x[:, None] - uN)  # weight -(x-u)
        upos = np.concatenate([uA, uB], 1)               # [16,12]
        vpos = np.concatenate([vA, vB], 1)

        cr = np.zeros((34, 384), np.float32)
        cr[0, 0:POSW] = upos.reshape(-1)
        cr[17, POSW:NCOL] = uN.reshape(-1)
        for i in range(SPC):
            cr[1 + i, PP * i:PP * (i + 1)] = 1.0
            cr[18 + i, POSW + NN * i:POSW + NN * (i + 1)] = 1.0
        for k, p in enumerate("pn"):
            w0, b0 = f32(p + "w0"), f32(p + "b0")
            base = NCOL
            cr[17 * k, base:base + 128] = w0[0]
            cr[17 * k + 1:17 * k + 17, base:base + 128] = (
                b0[None, :] + h.astype(np.float32) @ w0[1:, :])

        wsm = np.zeros((128, 408), np.float32)
        wsm[:, 0] = f32("pb1")
        wsm[:, 1] = f32("nb1")
        wsm[:, 2] = f32("cb1")
        wsm[0, 3] = f32("pb2")[0]
        wsm[0, 4] = f32("nb2")[0]
        wsm[0, 5] = f32("cb2")[0]
        wsm[0, 6] = f32("cb2")[1]
        wsm[0, 8:24] = 1.0
        wsm[1:33, 8:24] = h.T.astype(np.float32)
        wsm[0, 24:152] = f32("cb0")
        wsm[1:33, 24:152] = f32("cw0")
        wsm[0, 152:152 + POSW] = vpos.reshape(-1).astype(np.float32)
        wsm[0, 152 + POSW:152 + NCOL] = vN.reshape(-1).astype(np.float32)

        in_maps.append(dict(cr=cr, wbf=wbf, wsm=wsm))
    return in_maps


def kernel(**inputs):
    nc = _get_module()
    in_maps = make_in_maps(**inputs)
    res = run_bass_kernel_spmd(nc, in_maps, list(range(NCORES)))
    out = np.concatenate([res.results[c]["out"] for c in range(NCORES)], 0)
    return out.astype(np.float32)


if __name__ == "__main__":
    rng = np.random.default_rng(0)
    ins = dict(
        x=rng.random((B, 1), np.float32) * 2.0,
        h=rng.standard_normal((B, DH)).astype(np.float32),
    )
    for p in "pn":
        ins[p + "w0"] = rng.standard_normal((DH + 1, HID)).astype(np.float32) * 0.1
        ins[p + "b0"] = rng.standard_normal((HID,)).astype(np.float32) * 0.1
        ins[p + "w1"] = rng.standard_normal((HID, HID)).astype(np.float32) * 0.1
        ins[p + "b1"] = rng.standard_normal((HID,)).astype(np.float32) * 0.1
        ins[p + "w2"] = rng.standard_normal((HID, 1)).astype(np.float32) * 0.1
        ins[p + "b2"] = rng.standard_normal((1,)).astype(np.float32) * 0.1
    ins["cw0"] = rng.standard_normal((DH, HID)).astype(np.float32) * 0.1
    ins["cb0"] = rng.standard_normal((HID,)).astype(np.float32) * 0.1
    ins["cw1"] = rng.standard_normal((HID, HID)).astype(np.float32) * 0.1
    ins["cb1"] = rng.standard_normal((HID,)).astype(np.float32) * 0.1
    ins["cw2"] = rng.standard_normal((HID, 2)).astype(np.float32) * 0.1
    ins["cb2"] = rng.standard_normal((2,)).astype(np.float32) * 0.1
    print(kernel(**ins)[:4, 0])
